# revision 26
# baseline (speedup 1.0000x reference)
"""Multi-head attention (B=4, S=2048, D=1024, H=16) on 8 TRN2 NeuronCores.

Sharding: core c handles batch b = c//2 and head-group hg = c%2 (8 heads).
Tensor-parallel within the core pair of a batch: w_q/w_k/w_v column-split,
w_o row-split; host sums the two partial out-projections per batch.

Per-core schedule: K-projection (ch-major) + Q(p0,ch0) lead in; attention
runs chunk-major with V-projection, remaining Q-projections and the
out-projection issued as filler units interleaved into the attention
stream so the PE keeps busy while the exp engines work.

Softmax exp is split between the scalar engine (Exp LUT; pre-scale folded
into w_k) and the vector engine via a corrected Schraudolph bit trick:
tensor_scalar add + fp32->int16 convert writes bf16-exponent bits, then a
custom DVE op extracts the mantissa (bitwise AND/OR) and applies a cubic
correction u*(R3-v)*((v-A)^2+B); max rel err ~7e-3, rms ~2.3e-3.

attn@V uses stationary [V|1] per head (M=65; PSUM row 64 accumulates the
softmax denominator), both heads in one [128,1024] PSUM acc (bank per
head). The acc is copied once to SBUF, the denominator rows are
DMA-broadcast, one [64,1024] fast reciprocal + two muls produce the
normalized context; head B bounces to partitions 64..127 via SBUF DMA.
"""

import numpy as np
import ml_dtypes
from collections import deque
from contextlib import ExitStack

import concourse.bass as bass
import concourse.tile as tile
from concourse import bacc, mybir
from concourse.bass_utils import run_bass_kernel_spmd

BF16 = ml_dtypes.bfloat16
F32 = np.float32

D = 1024
N_HEAD = 16
DH = 64
HPC = 8          # heads per core
HW = HPC * DH    # head-group width = 512
HA = DH + 1      # per-head augmented width (V + ones col)
P = 128

# ---- corrected-Schraudolph exp constants (cubic fit of 2^(v-1)/v) ----
LOG2E = 1.4426950408889634
A_C = 1.0092522260509793
B_C = 2.6586870799825735
R3_C = 4.71326784746616
S_POS = 0.10114901850011002
C1_FOLD = 0.125 * LOG2E * 128.0                      # folded into w_k, b_k
C2_ADD = float(np.float32(16256.0 + 128.0 * np.log2(S_POS)))
LN2_128 = float(np.float32(1.0 / (128.0 * LOG2E)))   # ACT exp scale after fold
MASK_F32 = float(np.uint32(0x007F0000).view(np.float32))

# kt indices (of NT=16) whose exp runs on the DVE path; rest on ACT.
DVE_KT = frozenset((2, 5, 8, 11, 14))
MULS_ON_GPSIMD = True
BCAST_ON_PE = True

TRACE = False    # set by test.py for profiling runs

_PROG = {}


def _register_exp_fix():
    from concourse import dve_ops
    from concourse.dve_spec import (
        Spec, Src0, C0, C1, C2, C3, One, Bin, AluOp, lower,
        _spill_c3_to_src1, _has_src1,
    )
    from concourse.dve_uop import DveOpSpec

    for op in dve_ops.OPS:
        if op.name == "EXP_FIX_ANT_V2":
            return op

    vv = Bin(AluOp.BITWISE_OR, Bin(AluOp.BITWISE_AND, Src0, C3), One)
    d = vv - C0
    body = Src0 * ((C2 - vv) * (d * d + C1))

    def _ref(in0, in1, s0, s1, imm2):
        i = in0.astype(np.float32).view(np.int32)
        m = np.uint32(0x007F0000).view(np.int32)
        one = np.float32(1.0).view(np.int32)
        v = ((i & m) | one).view(np.float32)
        return in0 * ((imm2 - v) * ((v - s0) * (v - s0) + s1))

    op = dve_ops.DveOp(
        "EXP_FIX_ANT_V2",
        Spec(body=_spill_c3_to_src1(body), reference=_ref),
        subdim=False,
        uops_sha={},
    )
    dve_ops.OPS.append(op)
    dve_ops.CUSTOM_DVE_SPECS[op.name] = op.spec
    dve_ops._SUB_OPCODE_FOR_NAME[op.name] = (
        dve_ops._CUSTOM_DVE_ROW_BASE + len(dve_ops.OPS) - 1)
    for ver in ("v3", "v4"):
        spec_obj = DveOpSpec(
            name=op.name,
            opcode=dve_ops.get_dve_sub_opcode(op.name),
            uops=lower(op.spec, ver=ver),
            rd1_en=_has_src1(op.spec),
        )
        op.uops_sha[ver] = spec_obj.sha(ver)
    return op


def _bcast_dma(nc, dst, src_row, engine=None):
    """Broadcast a [1, W] SBUF row to [N, W] via a 0-stride partition DMA."""
    n = dst.shape[0]
    src_b = bass.AP(tensor=src_row.tensor, offset=src_row.offset,
                    ap=[list(src_row.ap[0]), [0, n], list(src_row.ap[1])])
    (engine or nc.sync).dma_start(dst, src_b)


def _view3(ap2d, n, w):
    """View a [P, n*w] AP as [P, n, w]."""
    return bass.AP(tensor=ap2d.tensor, offset=ap2d.offset,
                   ap=[list(ap2d.ap[0]), [w, n], [1, w]])


def _gather_rows(nc, dst3, src2d, ni, p, width, col_off, engine):
    """One DMA: dst3 [P, ni, width] <- src2d rows (i*p + part), cols
    col_off..col_off+width. Replaces ni separate dma_starts (descriptor
    generation on the sequencer is ~0.7us per dma_start)."""
    rs = src2d.ap().ap[0][0]   # row stride (elements) of the dram tensor
    src = bass.AP(tensor=src2d.ap().tensor,
                  offset=src2d.ap().offset + col_off,
                  ap=[[rs, p], [rs * p, ni], [1, width]])
    engine.dma_start(dst3, src)


def _build_program(S):
    dt = mybir.dt
    bf = dt.bfloat16
    f32 = dt.float32

    CH = 512                 # q-chunk width
    NCH = S // CH            # q-chunks
    NT = S // P              # k-tiles
    NI = D // P              # contraction tiles over model dim
    NP = HPC // 2            # head pairs
    NOO = D // P             # out-proj o-tiles
    NOQ = HW // P            # q/k-proj o-tiles (== NP)
    VH = HPC * HA // 2       # 260: half of the augmented V width
    AF = mybir.ActivationFunctionType

    exp_op = _register_exp_fix()

    nc = bacc.Bacc("TRN2", target_bir_lowering=False, debug=False)

    xq = nc.dram_tensor("xq", [D, S], bf, kind="ExternalInput")
    xk = nc.dram_tensor("xk", [D, S], bf, kind="ExternalInput")
    xv = nc.dram_tensor("xv", [D, S], bf, kind="ExternalInput")
    wq = nc.dram_tensor("wq", [D, HW], bf, kind="ExternalInput")
    wk = nc.dram_tensor("wk", [D, HW], bf, kind="ExternalInput")
    wv = nc.dram_tensor("wv", [D, HPC * HA], bf, kind="ExternalInput")
    bq = nc.dram_tensor("bq", [P, NOQ], f32, kind="ExternalInput")
    bk = nc.dram_tensor("bk", [P, NOQ], f32, kind="ExternalInput")
    bv = nc.dram_tensor("bv", [1, HPC * HA], f32, kind="ExternalInput")
    wo = nc.dram_tensor("wo", [HW, D], bf, kind="ExternalInput")
    yT = nc.dram_tensor("yT", [D, S], f32, kind="ExternalOutput")

    with tile.TileContext(nc) as tc:
        with ExitStack() as ctx:
            consts = ctx.enter_context(tc.tile_pool(name="consts", bufs=1))
            wpool = ctx.enter_context(tc.tile_pool(name="wpool", bufs=1))
            xkp = ctx.enter_context(tc.tile_pool(name="xkp", bufs=2))
            xqp = ctx.enter_context(tc.tile_pool(name="xqp", bufs=2))
            xvp = ctx.enter_context(tc.tile_pool(name="xvp", bufs=2))
            slabs = ctx.enter_context(tc.tile_pool(name="slabs", bufs=1))
            epool = ctx.enter_context(tc.tile_pool(name="epool", bufs=10))
            dpool = ctx.enter_context(tc.tile_pool(name="dpool", bufs=2))
            spool = ctx.enter_context(tc.tile_pool(name="spool", bufs=2))
            pspair = ctx.enter_context(
                tc.tile_pool(name="pspair", bufs=2, space="PSUM"))
            psacc = ctx.enter_context(
                tc.tile_pool(name="psacc", bufs=1, space="PSUM"))
            psfill = ctx.enter_context(
                tc.tile_pool(name="psfill", bufs=2, space="PSUM"))

            # ---- constants ----
            bq_sb = consts.tile([P, NOQ], f32)
            nc.sync.dma_start(bq_sb[:], bq.ap())
            bk_sb = consts.tile([P, NOQ], f32)
            nc.sync.dma_start(bk_sb[:], bk.ap())
            bv_row = consts.tile([1, HPC * HA], f32)
            nc.sync.dma_start(bv_row[:], bv.ap())
            vbias = consts.tile([P, HPC * HA], f32)
            _bcast_dma(nc, vbias[:], bv_row[0:1, :], engine=nc.gpsimd)
            mask_ap = consts.tile([P, 1], f32)
            nc.vector.memset(mask_ap[:], MASK_F32)
            ones64 = consts.tile([P, 64], f32)
            nc.vector.memset(ones64[:], 1.0)

            # ---- weights ----
            wq_sb = wpool.tile([P, NI, HW], bf)
            wk_sb = wpool.tile([P, NI, HW], bf)
            wv_sb = wpool.tile([P, NI, HPC * HA], bf)
            wo_sb = wpool.tile([P, NOQ, D], bf)

            # ---- persistent activation slabs ----
            q_slab = slabs.tile([P, NP, S], bf)
            k_slab = slabs.tile([P, NP, S], bf)
            v_sb = slabs.tile([P, NT, HPC, P], bf)
            nc.vector.memset(v_sb[:], 0.0)
            attn_sb = [slabs.tile([P, S], bf, name=f"attn{pp}")
                       for pp in range(NP)]

            # ---- per-chunk x tiles (pool rotation = 2 chunks in flight) ----
            xkc = [xkp.tile([P, NI, CH], bf, tag="xk", name=f"xk{c}")
                   for c in range(NCH)]
            xqc = [xqp.tile([P, NI, CH], bf, tag="xq", name=f"xq{c}")
                   for c in range(NCH)]
            xvc = [xvp.tile([P, NI, CH], bf, tag="xv", name=f"xv{c}")
                   for c in range(NCH)]

            def load_xc(tiles, x_dram, ch):
                _gather_rows(nc, tiles[ch][:], x_dram, NI, P, CH, ch * CH,
                             nc.sync)

            def load_w(w_sb, w_dram):
                _gather_rows(nc, w_sb[:], w_dram, w_sb.shape[1], P,
                             w_sb.shape[2], 0, nc.sync)

            # DMA issue order = priority order (single sync queue).
            _gather_rows(nc, wk_sb[:, :, 0:P], wk, NI, P, P, 0, nc.sync)
            load_xc(xkc, xk, 0)
            for o in range(1, NOQ):
                _gather_rows(nc, wk_sb[:, :, o * P:(o + 1) * P], wk, NI, P,
                             P, o * P, nc.sync)
            load_xc(xkc, xk, 1)
            load_w(wq_sb, wq)
            load_xc(xqc, xq, 0)
            load_w(wv_sb, wv)
            load_xc(xkc, xk, 2)      # aliases xk ch0 tiles: waits K(.,0) units
            load_xc(xkc, xk, 3)
            load_xc(xvc, xv, 0)
            load_xc(xvc, xv, 1)
            load_w(wo_sb, wo)

            # ---- unit issuers ----
            def qk_unit(w_sb, b_sb, slab, xc, o, ch):
                csl = slice(ch * CH, (ch + 1) * CH)
                fl = psfill.tile([P, CH], f32, tag="fill")
                for i in range(NI):
                    nc.tensor.matmul(fl[:], lhsT=w_sb[:, i, o * P:(o + 1) * P],
                                     rhs=xc[ch][:, i, :],
                                     start=(i == 0), stop=(i == NI - 1))
                nc.scalar.activation(slab[:, o, csl], fl[:],
                                     AF.Identity, bias=b_sb[:, o:o + 1])

            def v_unit(t):
                ch, tq = t // 4, t % 4
                tsl = slice(tq * P, (tq + 1) * P)
                for half in range(2):
                    vsl = slice(half * VH, (half + 1) * VH)
                    fl = psfill.tile([P, CH], f32, tag="fill")
                    for i in range(NI):
                        nc.tensor.matmul(fl[:, 0:VH],
                                         lhsT=xvc[ch][:, i, tsl],
                                         rhs=wv_sb[:, i, vsl],
                                         start=(i == 0), stop=(i == NI - 1))
                    nc.vector.tensor_add(
                        v_sb[:, t, half * 4:(half + 1) * 4, 0:HA],
                        _view3(fl[:, 0:VH], 4, HA),
                        _view3(vbias[:, vsl], 4, HA))

            def out_unit(o, ch):
                csl = slice(ch * CH, (ch + 1) * CH)
                fl = psfill.tile([P, CH], f32, tag="fill")
                for c in range(NOQ):
                    nc.tensor.matmul(fl[:], lhsT=wo_sb[:, c, o * P:(o + 1) * P],
                                     rhs=attn_sb[c][:, csl],
                                     start=(c == 0), stop=(c == NOQ - 1))
                st = spool.tile([P, CH], f32, tag="stage")
                nc.scalar.copy(st[:], fl[:])
                nc.gpsimd.dma_start(yT.ap()[o * P:(o + 1) * P, csl], st[:])

            # ---- filler machinery ----
            v_done = [False] * NT

            def ensure_v(t):
                for tt in range(t + 1):
                    if not v_done[tt]:
                        v_unit(tt)
                        v_done[tt] = True

            fillq = deque()
            issued = set()

            def push(key, fn):
                fillq.append((key, fn))

            def pull(n=1):
                for _ in range(n):
                    if fillq:
                        k, fn = fillq.popleft()
                        fn()
                        issued.add(k)

            def drain_until(key):
                while key not in issued:
                    assert fillq, f"filler underflow for {key}"
                    k, fn = fillq.popleft()
                    fn()
                    issued.add(k)

            # ---- attention for one (pair, chunk) ----
            # The normalize tail of iteration n (reciprocal + muls) is issued
            # mid-iteration n+1 so the denominator-broadcast DMA latency
            # (issued with the PSUM copy at n's end) hides behind n+1's exp
            # stream instead of bubbling the DVE queue.
            pending_fin = [None]

            def attention(p, ch, first, last):
                hA, hB = 2 * p, 2 * p + 1
                csl = slice(ch * CH, (ch + 1) * CH)
                acc = psacc.tile([P, 2 * CH], f32, tag="acc")
                dve_set = () if first else DVE_KT
                pend = []
                pull(3 if ch == NCH - 1 else 2)

                def issue_av(et, kt):
                    ensure_v(kt)
                    nc.tensor.matmul(
                        acc[:, 0:CH],
                        lhsT=v_sb[:, kt, hA, :],
                        rhs=et[:, 0:CH],
                        start=(kt == 0), stop=(kt == NT - 1))
                    nc.tensor.matmul(
                        acc[:, CH:2 * CH],
                        lhsT=v_sb[:, kt, hB, :],
                        rhs=et[:, CH:2 * CH],
                        start=(kt == 0), stop=(kt == NT - 1))

                for kt in range(NT):
                    ksl = slice(kt * P, (kt + 1) * P)
                    ps = pspair.tile([P, 1024], f32, tag="pair")
                    nc.tensor.matmul(
                        ps[:, 0:CH],
                        lhsT=k_slab[0:64, p, ksl],
                        rhs=q_slab[0:64, p, csl],
                        start=True, stop=True, tile_position=(0, 0))
                    nc.tensor.matmul(
                        ps[:, CH:2 * CH],
                        lhsT=k_slab[64:128, p, ksl],
                        rhs=q_slab[64:128, p, csl],
                        start=True, stop=True, tile_position=(64, 0))
                    et = epool.tile([P, 1024], bf, tag="exp")
                    if kt in dve_set:
                        nc.vector.tensor_scalar_add(
                            et[:].bitcast(dt.int16), ps[:], C2_ADD)
                        nc.vector._custom_dve(
                            exp_op, out=et[:], in0=et[:], in1=mask_ap[:],
                            s0=A_C, s1=B_C, imm2=R3_C)
                    else:
                        nc.scalar.activation(et[:], ps[:], AF.Exp,
                                             scale=LN2_128)
                    pend.append((et, kt))
                    if kt == 2 and pending_fin[0] is not None:
                        lps_prev = pending_fin[0][0]()
                    if kt == 3 and pending_fin[0] is not None:
                        pending_fin[0][1](lps_prev)
                        pending_fin[0] = None
                    if not first and kt in (5, 8, 11, 14):
                        if ch >= NCH - 2 or kt == 8:
                            pull(2 if ch == NCH - 1 else 1)
                    if len(pend) >= 8:
                        e0, k0 = pend.pop(0)
                        issue_av(e0, k0)
                for e0, k0 in pend:
                    issue_av(e0, k0)

                # normalize head: one PSUM->SBUF copy (both heads +
                # denominators, releases the acc banks).
                cp = dpool.tile([P, 2 * CH], f32, tag="cp")
                nc.vector.tensor_copy(cp[0:HA, :], acc[0:HA, :])
                nc.vector.tensor_copy(cp[96:97, CH:2 * CH],
                                      acc[DH:DH + 1, CH:2 * CH])

                # allocate now (pool order: right after acc) so the next
                # iteration's acc allocation lands behind it; issue later.
                lps = psacc.tile([P, 2 * CH], f32, tag="acc", name="lps")
                if not BCAST_ON_PE:
                    lbtd = dpool.tile([P, 2 * CH], f32, tag="lbtd")
                    _bcast_dma(nc, lbtd[0:64, :], cp[DH:DH + 1, :],
                               engine=nc.scalar)

                def bcast_mm():
                    if not BCAST_ON_PE:
                        return lbtd
                    # rank-1 PE matmuls broadcast the denominator rows into
                    # the acc banks freed by the cp copy; rows 64 and 96 are
                    # different row groups, so the two matmuls overlap
                    nc.tensor.matmul(lps[0:64, 0:CH],
                                     lhsT=ones64[DH:DH + 1, :],
                                     rhs=cp[DH:DH + 1, 0:CH],
                                     start=True, stop=True)
                    nc.tensor.matmul(lps[0:64, CH:2 * CH],
                                     lhsT=ones64[96:97, :],
                                     rhs=cp[96:97, CH:2 * CH],
                                     start=True, stop=True,
                                     tile_position=(96, 0))
                    return lps

                def finish(lps):
                    # PSUM->SBUF reciprocal + muls; head B bounced by DMA
                    lbt = dpool.tile([P, 2 * CH], f32, tag="lbt")
                    nc.vector.reciprocal_approx_fast(lbt[0:64, :],
                                                     lps[0:64, :])
                    eng = nc.gpsimd if MULS_ON_GPSIMD else nc.vector
                    eng.tensor_mul(attn_sb[p][0:64, csl], cp[0:64, 0:CH],
                                   lbt[0:64, 0:CH])
                    tmpb = dpool.tile([P, CH], bf, tag="tmpb")
                    eng.tensor_mul(tmpb[0:64, :], cp[0:64, CH:2 * CH],
                                   lbt[0:64, CH:2 * CH])
                    nc.gpsimd.dma_start(attn_sb[p][64:128, csl], tmpb[0:64, :])
                    # out-units for this chunk become eligible only once all
                    # its attn_sb writes are issued (dependency-tracking)
                    if p == NP - 1 and ch < NCH - 1:
                        for o in range(NOO):
                            push(("out", o, ch),
                                 lambda o=o: out_unit(o, ch))

                if last:
                    finish(bcast_mm())
                else:
                    pending_fin[0] = (bcast_mm, finish)

            # ---- lead-in: all K units (ch-major), then Q(p0, ch0) ----
            for ch in range(NCH):
                for o in range(NOQ):
                    qk_unit(wk_sb, bk_sb, k_slab, xkc, o, ch)
            qk_unit(wq_sb, bq_sb, q_slab, xqc, 0, 0)
            issued.add(("Q", 0, 0))

            for p in range(1, NP):
                push(("Q", p, 0),
                     lambda p=p: qk_unit(wq_sb, bq_sb, q_slab, xqc, p, 0))

            # ---- main loop: chunk-major attention ----
            xq_loaded = [True, False, False, False]
            for ch in range(NCH):
                for p in range(NP):
                    if ch == 0 and p == 0:
                        load_xc(xvc, xv, 2)
                        load_xc(xvc, xv, 3)
                    if p == 1 and ch < NCH - 1 and not xq_loaded[ch + 1]:
                        load_xc(xqc, xq, ch + 1)
                        xq_loaded[ch + 1] = True
                    if (p, ch) != (0, 0):
                        drain_until(("Q", p, ch))
                    attention(p, ch, first=(ch == 0 and p == 0),
                              last=(ch == NCH - 1 and p == NP - 1))
                    if ch < NCH - 1:
                        push(("Q", p, ch + 1),
                             lambda p=p, ch=ch: qk_unit(
                                 wq_sb, bq_sb, q_slab, xqc, p, ch + 1))
                if ch == NCH - 1:
                    pull(len(fillq))
                    for o in range(NOO):
                        out_unit(o, ch)

    nc.compile()
    return nc


def _get_program(S):
    if S not in _PROG:
        _PROG[S] = _build_program(S)
    return _PROG[S]


def enable_trace():
    """Register the NTFF profiling hook (axon images lack antenv.axon_hooks)
    and neuter the cloud artifact upload; then TRACE=True runs return
    exec_time_ns."""
    global TRACE
    import sys
    import types
    import antenv
    if "antenv.axon_hooks" not in sys.modules:
        _m = types.ModuleType("antenv.axon_hooks")
        _m._hook = None
        _m.set_axon_ntff_profile_hook = lambda h: setattr(_m, "_hook", h)
        _m.get_axon_ntff_profile_hook = lambda: _m._hook
        sys.modules["antenv.axon_hooks"] = _m
        antenv.axon_hooks = _m
        from trn_agent_boot.trn_boot import _ntff_profile_via_ctypes
        _m._hook = _ntff_profile_via_ctypes("/opt/axon/libaxon_pjrt.so")
    import concourse.bass_utils as bu
    bu.upload_artifacts = lambda tmpdir: tmpdir
    TRACE = True


def _prep_core_inputs(q, k, v, w_q, b_q, w_k, b_k, w_v, b_v, b, hg, S):
    hsl = slice(hg * HW, (hg + 1) * HW)
    wv_aug = np.zeros((D, HPC * HA), F32)
    bv_aug = np.zeros((1, HPC * HA), F32)
    wv_s = w_v[hsl]
    bv_s = b_v[hsl]
    for h in range(HPC):
        wv_aug[:, h * HA:h * HA + DH] = wv_s[h * DH:(h + 1) * DH].T
        bv_aug[0, h * HA:h * HA + DH] = bv_s[h * DH:(h + 1) * DH]
        bv_aug[0, h * HA + DH] = 1.0
    return {
        "xq": np.ascontiguousarray(q[b].T).astype(BF16),
        "xk": np.ascontiguousarray(k[b].T).astype(BF16),
        "xv": np.ascontiguousarray(v[b].T).astype(BF16),
        "wq": np.ascontiguousarray(w_q[hsl].T).astype(BF16),
        "wk": np.ascontiguousarray(w_k[hsl].T * F32(C1_FOLD)).astype(BF16),
        "wv": wv_aug.astype(BF16),
        "bq": np.ascontiguousarray(b_q[hsl].reshape(HW // P, P).T).astype(F32),
        "bk": np.ascontiguousarray(
            (b_k[hsl] * F32(C1_FOLD)).reshape(HW // P, P).T).astype(F32),
        "bv": bv_aug,
    }


def kernel(q, k, v, w_q, b_q, w_k, b_k, w_v, b_v, w_o, b_o):
    q, k, v = (np.asarray(a, F32) for a in (q, k, v))
    w_q, b_q, w_k, b_k = (np.asarray(a, F32) for a in (w_q, b_q, w_k, b_k))
    w_v, b_v, w_o, b_o = (np.asarray(a, F32) for a in (w_v, b_v, w_o, b_o))
    B, S, _ = q.shape

    nc = _get_program(S)

    n_cores = 2 * B
    in_maps = []
    for c in range(n_cores):
        b, hg = c // 2, c % 2
        m = _prep_core_inputs(q, k, v, w_q, b_q, w_k, b_k, w_v, b_v, b, hg, S)
        hsl = slice(hg * HW, (hg + 1) * HW)
        m["wo"] = np.ascontiguousarray(w_o[:, hsl].T).astype(BF16)
        in_maps.append(m)

    res = run_bass_kernel_spmd(nc, in_maps, list(range(n_cores)), trace=TRACE)

    out = np.empty((B, S, D), F32)
    for b in range(B):
        yt = res.results[2 * b]["yT"] + res.results[2 * b + 1]["yT"]
        out[b] = yt.T + b_o
    if TRACE:
        kernel.last_exec_time_ns = res.exec_time_ns
    return out


# revision 27
# speedup vs baseline: 1.1813x; 1.1813x over previous
"""Multi-head attention (B=4, S=2048, D=1024, H=16) on 8 TRN2 NeuronCores.

Sharding: core c handles batch b = c//2 and head-group hg = c%2 (8 heads).
Tensor-parallel within the core pair of a batch: w_q/w_k/w_v column-split,
w_o row-split; host sums the two partial out-projections per batch.

Per-core schedule: K-projection (ch-major) + Q(p0,ch0) lead in; attention
runs chunk-major with V-projection, remaining Q-projections and the
out-projection issued as filler units interleaved into the attention
stream so the PE keeps busy while the exp engines work.

Softmax exp is split between the scalar engine (Exp LUT; pre-scale folded
into w_k) and the vector engine via a corrected Schraudolph bit trick:
tensor_scalar add + fp32->int16 convert writes bf16-exponent bits, then a
custom DVE op extracts the mantissa (bitwise AND/OR) and applies a cubic
correction u*(R3-v)*((v-A)^2+B); max rel err ~7e-3, rms ~2.3e-3.

attn@V uses stationary [V|1] per head (M=65; PSUM row 64 accumulates the
softmax denominator), both heads in one [128,1024] PSUM acc (bank per
head). The acc is copied once to SBUF, the denominator rows are
DMA-broadcast, one [64,1024] fast reciprocal + two muls produce the
normalized context; head B bounces to partitions 64..127 via SBUF DMA.
"""

import numpy as np
import ml_dtypes
from collections import deque
from contextlib import ExitStack

import concourse.bass as bass
import concourse.tile as tile
from concourse import bacc, mybir
from concourse.bass_utils import run_bass_kernel_spmd

BF16 = ml_dtypes.bfloat16
F32 = np.float32

D = 1024
N_HEAD = 16
DH = 64
HPC = 8          # heads per core
HW = HPC * DH    # head-group width = 512
HA = DH + 1      # per-head augmented width (V + ones col)
P = 128

# ---- corrected-Schraudolph exp constants (cubic fit of 2^(v-1)/v) ----
LOG2E = 1.4426950408889634
A_C = 1.0092522260509793
B_C = 2.6586870799825735
R3_C = 4.71326784746616
S_POS = 0.10114901850011002
C1_FOLD = 0.125 * LOG2E * 128.0                      # folded into w_k, b_k
C2_ADD = float(np.float32(16256.0 + 128.0 * np.log2(S_POS)))
LN2_128 = float(np.float32(1.0 / (128.0 * LOG2E)))   # ACT exp scale after fold
MASK_F32 = float(np.uint32(0x007F0000).view(np.float32))

# kt indices (of NT=16) whose exp runs on the DVE path; rest on ACT.
DVE_KT = frozenset((2, 5, 8, 11, 14))
MULS_ON_GPSIMD = True
BCAST_ON_PE = True

TRACE = False    # set by test.py for profiling runs

_PROG = {}


def _register_exp_fix():
    from concourse import dve_ops
    from concourse.dve_spec import (
        Spec, Src0, C0, C1, C2, C3, One, Bin, AluOp, lower,
        _spill_c3_to_src1, _has_src1,
    )
    from concourse.dve_uop import DveOpSpec

    for op in dve_ops.OPS:
        if op.name == "EXP_FIX_ANT_V2":
            return op

    vv = Bin(AluOp.BITWISE_OR, Bin(AluOp.BITWISE_AND, Src0, C3), One)
    d = vv - C0
    body = Src0 * ((C2 - vv) * (d * d + C1))

    def _ref(in0, in1, s0, s1, imm2):
        i = in0.astype(np.float32).view(np.int32)
        m = np.uint32(0x007F0000).view(np.int32)
        one = np.float32(1.0).view(np.int32)
        v = ((i & m) | one).view(np.float32)
        return in0 * ((imm2 - v) * ((v - s0) * (v - s0) + s1))

    op = dve_ops.DveOp(
        "EXP_FIX_ANT_V2",
        Spec(body=_spill_c3_to_src1(body), reference=_ref),
        subdim=False,
        uops_sha={},
    )
    dve_ops.OPS.append(op)
    dve_ops.CUSTOM_DVE_SPECS[op.name] = op.spec
    dve_ops._SUB_OPCODE_FOR_NAME[op.name] = (
        dve_ops._CUSTOM_DVE_ROW_BASE + len(dve_ops.OPS) - 1)
    for ver in ("v3", "v4"):
        spec_obj = DveOpSpec(
            name=op.name,
            opcode=dve_ops.get_dve_sub_opcode(op.name),
            uops=lower(op.spec, ver=ver),
            rd1_en=_has_src1(op.spec),
        )
        op.uops_sha[ver] = spec_obj.sha(ver)
    return op


def _bcast_dma(nc, dst, src_row, engine=None):
    """Broadcast a [1, W] SBUF row to [N, W] via a 0-stride partition DMA."""
    n = dst.shape[0]
    src_b = bass.AP(tensor=src_row.tensor, offset=src_row.offset,
                    ap=[list(src_row.ap[0]), [0, n], list(src_row.ap[1])])
    (engine or nc.sync).dma_start(dst, src_b)


def _view3(ap2d, n, w):
    """View a [P, n*w] AP as [P, n, w]."""
    return bass.AP(tensor=ap2d.tensor, offset=ap2d.offset,
                   ap=[list(ap2d.ap[0]), [w, n], [1, w]])


def _gather_rows(nc, dst3, src2d, ni, p, width, col_off, engine):
    """One DMA: dst3 [P, ni, width] <- src2d rows (i*p + part), cols
    col_off..col_off+width. Replaces ni separate dma_starts (descriptor
    generation on the sequencer is ~0.7us per dma_start)."""
    rs = src2d.ap().ap[0][0]   # row stride (elements) of the dram tensor
    src = bass.AP(tensor=src2d.ap().tensor,
                  offset=src2d.ap().offset + col_off,
                  ap=[[rs, p], [rs * p, ni], [1, width]])
    engine.dma_start(dst3, src)


def _build_program(S):
    dt = mybir.dt
    bf = dt.bfloat16
    f32 = dt.float32

    CH = 512                 # q-chunk width
    NCH = S // CH            # q-chunks
    NT = S // P              # k-tiles
    NI = D // P              # contraction tiles over model dim
    NP = HPC // 2            # head pairs
    NOO = D // P             # out-proj o-tiles
    NOQ = HW // P            # q/k-proj o-tiles (== NP)
    VH = HPC * HA // 2       # 260: half of the augmented V width
    AF = mybir.ActivationFunctionType

    exp_op = _register_exp_fix()

    nc = bacc.Bacc("TRN2", target_bir_lowering=False, debug=False)

    xq = nc.dram_tensor("xq", [D, S], bf, kind="ExternalInput")
    xk = nc.dram_tensor("xk", [D, S], bf, kind="ExternalInput")
    xv = nc.dram_tensor("xv", [D, S], bf, kind="ExternalInput")
    wq = nc.dram_tensor("wq", [D, HW], bf, kind="ExternalInput")
    wk = nc.dram_tensor("wk", [D, HW], bf, kind="ExternalInput")
    wv = nc.dram_tensor("wv", [D, HPC * HA], bf, kind="ExternalInput")
    bq = nc.dram_tensor("bq", [P, NOQ], f32, kind="ExternalInput")
    bk = nc.dram_tensor("bk", [P, NOQ], f32, kind="ExternalInput")
    bv = nc.dram_tensor("bv", [1, HPC * HA], f32, kind="ExternalInput")
    wo = nc.dram_tensor("wo", [HW, D], bf, kind="ExternalInput")
    yT = nc.dram_tensor("yT", [D, S], f32, kind="ExternalOutput")

    with tile.TileContext(nc) as tc:
        with ExitStack() as ctx:
            consts = ctx.enter_context(tc.tile_pool(name="consts", bufs=1))
            wpool = ctx.enter_context(tc.tile_pool(name="wpool", bufs=1))
            xkp = ctx.enter_context(tc.tile_pool(name="xkp", bufs=2))
            xqp = ctx.enter_context(tc.tile_pool(name="xqp", bufs=2))
            xvp = ctx.enter_context(tc.tile_pool(name="xvp", bufs=2))
            slabs = ctx.enter_context(tc.tile_pool(name="slabs", bufs=1))
            epool = ctx.enter_context(tc.tile_pool(name="epool", bufs=10))
            dpool = ctx.enter_context(tc.tile_pool(name="dpool", bufs=2))
            spool = ctx.enter_context(tc.tile_pool(name="spool", bufs=2))
            pspair = ctx.enter_context(
                tc.tile_pool(name="pspair", bufs=2, space="PSUM"))
            psacc = ctx.enter_context(
                tc.tile_pool(name="psacc", bufs=1, space="PSUM"))
            psfill = ctx.enter_context(
                tc.tile_pool(name="psfill", bufs=2, space="PSUM"))

            # ---- constants ----
            bq_sb = consts.tile([P, NOQ], f32)
            nc.sync.dma_start(bq_sb[:], bq.ap())
            bk_sb = consts.tile([P, NOQ], f32)
            nc.sync.dma_start(bk_sb[:], bk.ap())
            bv_row = consts.tile([1, HPC * HA], f32)
            nc.sync.dma_start(bv_row[:], bv.ap())
            vbias = consts.tile([P, HPC * HA], f32)
            _bcast_dma(nc, vbias[:], bv_row[0:1, :], engine=nc.gpsimd)
            mask_ap = consts.tile([P, 1], f32)
            nc.vector.memset(mask_ap[:], MASK_F32)
            ones64 = consts.tile([P, 64], f32)
            nc.vector.memset(ones64[:], 1.0)

            # ---- weights ----
            wq_sb = wpool.tile([P, NI, HW], bf)
            wk_sb = wpool.tile([P, NI, HW], bf)
            wv_sb = wpool.tile([P, NI, HPC * HA], bf)
            wo_sb = wpool.tile([P, NOQ, D], bf)

            # ---- persistent activation slabs ----
            q_slab = slabs.tile([P, NP, S], bf)
            k_slab = slabs.tile([P, NP, S], bf)
            v_sb = slabs.tile([P, NT, HPC * HA], bf)
            attn_sb = [slabs.tile([P, S], bf, name=f"attn{pp}")
                       for pp in range(NP)]

            # ---- per-chunk x tiles (pool rotation = 2 chunks in flight) ----
            xkc = [xkp.tile([P, NI, CH], bf, tag="xk", name=f"xk{c}")
                   for c in range(NCH)]
            xqc = [xqp.tile([P, NI, CH], bf, tag="xq", name=f"xq{c}")
                   for c in range(NCH)]
            xvc = [xvp.tile([P, NI, CH], bf, tag="xv", name=f"xv{c}")
                   for c in range(NCH)]

            def load_xc(tiles, x_dram, ch):
                _gather_rows(nc, tiles[ch][:], x_dram, NI, P, CH, ch * CH,
                             nc.sync)

            def load_w(w_sb, w_dram):
                _gather_rows(nc, w_sb[:], w_dram, w_sb.shape[1], P,
                             w_sb.shape[2], 0, nc.sync)

            # DMA issue order = priority order (single sync queue).
            _gather_rows(nc, wk_sb[:, :, 0:P], wk, NI, P, P, 0, nc.sync)
            load_xc(xkc, xk, 0)
            for o in range(1, NOQ):
                _gather_rows(nc, wk_sb[:, :, o * P:(o + 1) * P], wk, NI, P,
                             P, o * P, nc.sync)
            load_xc(xkc, xk, 1)
            load_w(wq_sb, wq)
            load_xc(xqc, xq, 0)
            load_w(wv_sb, wv)
            load_xc(xkc, xk, 2)      # aliases xk ch0 tiles: waits K(.,0) units
            load_xc(xkc, xk, 3)
            load_xc(xvc, xv, 0)
            load_xc(xvc, xv, 1)
            load_w(wo_sb, wo)

            # ---- unit issuers ----
            def qk_unit(w_sb, b_sb, slab, xc, o, ch):
                csl = slice(ch * CH, (ch + 1) * CH)
                fl = psfill.tile([P, CH], f32, tag="fill")
                for i in range(NI):
                    nc.tensor.matmul(fl[:], lhsT=w_sb[:, i, o * P:(o + 1) * P],
                                     rhs=xc[ch][:, i, :],
                                     start=(i == 0), stop=(i == NI - 1))
                nc.scalar.activation(slab[:, o, csl], fl[:],
                                     AF.Identity, bias=b_sb[:, o:o + 1])

            def v_unit(t):
                ch, tq = t // 4, t % 4
                tsl = slice(tq * P, (tq + 1) * P)
                for half in range(2):
                    vsl = slice(half * VH, (half + 1) * VH)
                    fl = psfill.tile([P, CH], f32, tag="fill")
                    for i in range(NI):
                        nc.tensor.matmul(fl[:, 0:VH],
                                         lhsT=xvc[ch][:, i, tsl],
                                         rhs=wv_sb[:, i, vsl],
                                         start=(i == 0), stop=(i == NI - 1))
                    nc.vector.tensor_add(v_sb[:, t, vsl], fl[:, 0:VH],
                                         vbias[:, vsl])

            def out_unit(o, ch):
                csl = slice(ch * CH, (ch + 1) * CH)
                fl = psfill.tile([P, CH], f32, tag="fill")
                for c in range(NOQ):
                    nc.tensor.matmul(fl[:], lhsT=wo_sb[:, c, o * P:(o + 1) * P],
                                     rhs=attn_sb[c][:, csl],
                                     start=(c == 0), stop=(c == NOQ - 1))
                st = spool.tile([P, CH], f32, tag="stage")
                nc.scalar.copy(st[:], fl[:])
                nc.gpsimd.dma_start(yT.ap()[o * P:(o + 1) * P, csl], st[:])

            # ---- filler machinery ----
            v_done = [False] * NT

            def ensure_v(t):
                for tt in range(t + 1):
                    if not v_done[tt]:
                        v_unit(tt)
                        v_done[tt] = True

            fillq = deque()
            issued = set()

            def push(key, fn):
                fillq.append((key, fn))

            def pull(n=1):
                for _ in range(n):
                    if fillq:
                        k, fn = fillq.popleft()
                        fn()
                        issued.add(k)

            def drain_until(key):
                while key not in issued:
                    assert fillq, f"filler underflow for {key}"
                    k, fn = fillq.popleft()
                    fn()
                    issued.add(k)

            # ---- attention for one (pair, chunk) ----
            # The normalize tail of iteration n (reciprocal + muls) is issued
            # mid-iteration n+1 so the denominator-broadcast DMA latency
            # (issued with the PSUM copy at n's end) hides behind n+1's exp
            # stream instead of bubbling the DVE queue.
            pending_fin = [None]

            def attention(p, ch, first, last):
                hA, hB = 2 * p, 2 * p + 1
                csl = slice(ch * CH, (ch + 1) * CH)
                acc = psacc.tile([P, 2 * CH], f32, tag="acc")
                dve_set = () if first else DVE_KT
                pend = []
                pull(3 if ch == NCH - 1 else 2)

                def issue_av(et, kt):
                    ensure_v(kt)
                    nc.tensor.matmul(
                        acc[0:HA, 0:CH],
                        lhsT=v_sb[:, kt, hA * HA:(hA + 1) * HA],
                        rhs=et[:, 0:CH],
                        start=(kt == 0), stop=(kt == NT - 1))
                    nc.tensor.matmul(
                        acc[0:HA, CH:2 * CH],
                        lhsT=v_sb[:, kt, hB * HA:(hB + 1) * HA],
                        rhs=et[:, CH:2 * CH],
                        start=(kt == 0), stop=(kt == NT - 1))

                for kt in range(NT):
                    ksl = slice(kt * P, (kt + 1) * P)
                    ps = pspair.tile([P, 1024], f32, tag="pair")
                    nc.tensor.matmul(
                        ps[:, 0:CH],
                        lhsT=k_slab[0:64, p, ksl],
                        rhs=q_slab[0:64, p, csl],
                        start=True, stop=True, tile_position=(0, 0))
                    nc.tensor.matmul(
                        ps[:, CH:2 * CH],
                        lhsT=k_slab[64:128, p, ksl],
                        rhs=q_slab[64:128, p, csl],
                        start=True, stop=True, tile_position=(64, 0))
                    et = epool.tile([P, 1024], bf, tag="exp")
                    if kt in dve_set:
                        nc.vector.tensor_scalar_add(
                            et[:].bitcast(dt.int16), ps[:], C2_ADD)
                        nc.vector._custom_dve(
                            exp_op, out=et[:], in0=et[:], in1=mask_ap[:],
                            s0=A_C, s1=B_C, imm2=R3_C)
                    else:
                        nc.scalar.activation(et[:], ps[:], AF.Exp,
                                             scale=LN2_128)
                    pend.append((et, kt))
                    if kt == 2 and pending_fin[0] is not None:
                        lps_prev = pending_fin[0][0]()
                    if kt == 3 and pending_fin[0] is not None:
                        pending_fin[0][1](lps_prev)
                        pending_fin[0] = None
                    if not first and kt in (5, 8, 11, 14):
                        if ch >= NCH - 2 or kt == 8:
                            pull(2 if ch == NCH - 1 else 1)
                    if len(pend) >= 8:
                        e0, k0 = pend.pop(0)
                        issue_av(e0, k0)
                for e0, k0 in pend:
                    issue_av(e0, k0)

                # normalize head: one PSUM->SBUF copy (both heads +
                # denominators, releases the acc banks).
                cp = dpool.tile([P, 2 * CH], f32, tag="cp")
                nc.vector.tensor_copy(cp[0:HA, :], acc[0:HA, :])
                nc.vector.tensor_copy(cp[96:97, CH:2 * CH],
                                      acc[DH:DH + 1, CH:2 * CH])

                # allocate now (pool order: right after acc) so the next
                # iteration's acc allocation lands behind it; issue later.
                lps = psacc.tile([P, 2 * CH], f32, tag="acc", name="lps")
                if not BCAST_ON_PE:
                    lbtd = dpool.tile([P, 2 * CH], f32, tag="lbtd")
                    _bcast_dma(nc, lbtd[0:64, :], cp[DH:DH + 1, :],
                               engine=nc.scalar)

                def bcast_mm():
                    if not BCAST_ON_PE:
                        return lbtd
                    # rank-1 PE matmuls broadcast the denominator rows into
                    # the acc banks freed by the cp copy; rows 64 and 96 are
                    # different row groups, so the two matmuls overlap
                    nc.tensor.matmul(lps[0:64, 0:CH],
                                     lhsT=ones64[DH:DH + 1, :],
                                     rhs=cp[DH:DH + 1, 0:CH],
                                     start=True, stop=True)
                    nc.tensor.matmul(lps[0:64, CH:2 * CH],
                                     lhsT=ones64[96:97, :],
                                     rhs=cp[96:97, CH:2 * CH],
                                     start=True, stop=True,
                                     tile_position=(96, 0))
                    return lps

                def finish(lps):
                    # PSUM->SBUF reciprocal + muls; head B bounced by DMA
                    lbt = dpool.tile([P, 2 * CH], f32, tag="lbt")
                    nc.vector.reciprocal_approx_fast(lbt[0:64, :],
                                                     lps[0:64, :])
                    eng = nc.gpsimd if MULS_ON_GPSIMD else nc.vector
                    eng.tensor_mul(attn_sb[p][0:64, csl], cp[0:64, 0:CH],
                                   lbt[0:64, 0:CH])
                    tmpb = dpool.tile([P, CH], bf, tag="tmpb")
                    eng.tensor_mul(tmpb[0:64, :], cp[0:64, CH:2 * CH],
                                   lbt[0:64, CH:2 * CH])
                    nc.gpsimd.dma_start(attn_sb[p][64:128, csl], tmpb[0:64, :])
                    # out-units for this chunk become eligible only once all
                    # its attn_sb writes are issued (dependency-tracking)
                    if p == NP - 1 and ch < NCH - 1:
                        for o in range(NOO):
                            push(("out", o, ch),
                                 lambda o=o: out_unit(o, ch))

                if last:
                    finish(bcast_mm())
                else:
                    pending_fin[0] = (bcast_mm, finish)

            # ---- lead-in: all K units (ch-major), then Q(p0, ch0) ----
            for ch in range(NCH):
                for o in range(NOQ):
                    qk_unit(wk_sb, bk_sb, k_slab, xkc, o, ch)
            qk_unit(wq_sb, bq_sb, q_slab, xqc, 0, 0)
            issued.add(("Q", 0, 0))

            for p in range(1, NP):
                push(("Q", p, 0),
                     lambda p=p: qk_unit(wq_sb, bq_sb, q_slab, xqc, p, 0))

            # ---- main loop: chunk-major attention ----
            xq_loaded = [True, False, False, False]
            for ch in range(NCH):
                for p in range(NP):
                    if ch == 0 and p == 0:
                        load_xc(xvc, xv, 2)
                        load_xc(xvc, xv, 3)
                    if p == 1 and ch < NCH - 1 and not xq_loaded[ch + 1]:
                        load_xc(xqc, xq, ch + 1)
                        xq_loaded[ch + 1] = True
                    if (p, ch) != (0, 0):
                        drain_until(("Q", p, ch))
                    attention(p, ch, first=(ch == 0 and p == 0),
                              last=(ch == NCH - 1 and p == NP - 1))
                    if ch < NCH - 1:
                        push(("Q", p, ch + 1),
                             lambda p=p, ch=ch: qk_unit(
                                 wq_sb, bq_sb, q_slab, xqc, p, ch + 1))
                if ch == NCH - 1:
                    pull(len(fillq))
                    for o in range(NOO):
                        out_unit(o, ch)

    nc.compile()
    return nc


def _get_program(S):
    if S not in _PROG:
        _PROG[S] = _build_program(S)
    return _PROG[S]


def enable_trace():
    """Register the NTFF profiling hook (axon images lack antenv.axon_hooks)
    and neuter the cloud artifact upload; then TRACE=True runs return
    exec_time_ns."""
    global TRACE
    import sys
    import types
    import antenv
    if "antenv.axon_hooks" not in sys.modules:
        _m = types.ModuleType("antenv.axon_hooks")
        _m._hook = None
        _m.set_axon_ntff_profile_hook = lambda h: setattr(_m, "_hook", h)
        _m.get_axon_ntff_profile_hook = lambda: _m._hook
        sys.modules["antenv.axon_hooks"] = _m
        antenv.axon_hooks = _m
        from trn_agent_boot.trn_boot import _ntff_profile_via_ctypes
        _m._hook = _ntff_profile_via_ctypes("/opt/axon/libaxon_pjrt.so")
    import concourse.bass_utils as bu
    bu.upload_artifacts = lambda tmpdir: tmpdir
    TRACE = True


def _prep_core_inputs(q, k, v, w_q, b_q, w_k, b_k, w_v, b_v, b, hg, S):
    hsl = slice(hg * HW, (hg + 1) * HW)
    wv_aug = np.zeros((D, HPC * HA), F32)
    bv_aug = np.zeros((1, HPC * HA), F32)
    wv_s = w_v[hsl]
    bv_s = b_v[hsl]
    for h in range(HPC):
        wv_aug[:, h * HA:h * HA + DH] = wv_s[h * DH:(h + 1) * DH].T
        bv_aug[0, h * HA:h * HA + DH] = bv_s[h * DH:(h + 1) * DH]
        bv_aug[0, h * HA + DH] = 1.0
    return {
        "xq": np.ascontiguousarray(q[b].T).astype(BF16),
        "xk": np.ascontiguousarray(k[b].T).astype(BF16),
        "xv": np.ascontiguousarray(v[b].T).astype(BF16),
        "wq": np.ascontiguousarray(w_q[hsl].T).astype(BF16),
        "wk": np.ascontiguousarray(w_k[hsl].T * F32(C1_FOLD)).astype(BF16),
        "wv": wv_aug.astype(BF16),
        "bq": np.ascontiguousarray(b_q[hsl].reshape(HW // P, P).T).astype(F32),
        "bk": np.ascontiguousarray(
            (b_k[hsl] * F32(C1_FOLD)).reshape(HW // P, P).T).astype(F32),
        "bv": bv_aug,
    }


def kernel(q, k, v, w_q, b_q, w_k, b_k, w_v, b_v, w_o, b_o):
    q, k, v = (np.asarray(a, F32) for a in (q, k, v))
    w_q, b_q, w_k, b_k = (np.asarray(a, F32) for a in (w_q, b_q, w_k, b_k))
    w_v, b_v, w_o, b_o = (np.asarray(a, F32) for a in (w_v, b_v, w_o, b_o))
    B, S, _ = q.shape

    nc = _get_program(S)

    n_cores = 2 * B
    in_maps = []
    for c in range(n_cores):
        b, hg = c // 2, c % 2
        m = _prep_core_inputs(q, k, v, w_q, b_q, w_k, b_k, w_v, b_v, b, hg, S)
        hsl = slice(hg * HW, (hg + 1) * HW)
        m["wo"] = np.ascontiguousarray(w_o[:, hsl].T).astype(BF16)
        in_maps.append(m)

    res = run_bass_kernel_spmd(nc, in_maps, list(range(n_cores)), trace=TRACE)

    out = np.empty((B, S, D), F32)
    for b in range(B):
        yt = res.results[2 * b]["yT"] + res.results[2 * b + 1]["yT"]
        out[b] = yt.T + b_o
    if TRACE:
        kernel.last_exec_time_ns = res.exec_time_ns
    return out
